# revision 13
# baseline (speedup 1.0000x reference)
"""Trainium2 Bass kernel for nn_MultiHeadAttention (B=4, S=2048, D=1024, H=16).

Sharding: 8 cores = 4 batches x 2 query-halves. Each core computes full K/V
projections for its batch (keys permuted so the core's own half comes first),
attention for its 1024 queries over all 2048 keys, and the output projection
for its query half. No collectives.

Per-core dataflow (bf16 matmuls, fp32 PSUM accumulation):
  xT arrives pre-transposed from the host ([D, S] bf16), so there is no
  on-device transpose phase. Attention runs ACT(exp)-paced, (q-span outer,
  head-pair inner): per block, 16 key chunks of row-paired score matmuls ->
  exp -> ones-augmented AV matmuls. All projection work (qT/kT/v/output) is
  drip-fed just-in-time into the PE's slack via a deadline-ordered work
  queue, so the scalar engine starts exp'ing within ~10us of launch. Softmax
  denominator rides in the augmented V column; the epilogue drains PSUM to
  SBUF immediately (to unblock the next block's AV accumulation), then does
  reciprocal + K=1 broadcast matmul + bias. The output projection for the
  first q-half overlaps the second half's attention.
"""

import numpy as np
import ml_dtypes
from collections import deque
from contextlib import ExitStack

import concourse.bass as bass
from concourse import bacc
import concourse.mybir as mybir
import concourse.tile as tile
from concourse.bass_utils import run_bass_kernel_spmd

F32 = mybir.dt.float32
BF16 = mybir.dt.bfloat16
AF = mybir.ActivationFunctionType
NPBF16 = ml_dtypes.bfloat16

P = 128

N_CORES = 8
B_FULL, S_FULL, D_FULL = 4, 2048, 1024
H_FULL, DH = 16, 64
W65 = DH + 1
QLET = "ABCD"


def build_mha_nc(S=2048, Sq=1024, D=1024, H=16, scale=None):
    assert D % P == 0 and S % P == 0 and Sq % P == 0 and H % 2 == 0
    ND = D // P            # d-chunks (also number of head pairs)
    NS = S // P            # key chunks
    NPAIR = H // 2         # 8 head pairs
    QSP = 512              # q span per attention block
    NQS = Sq // QSP        # 2
    KSP = 512              # kT projection span
    NKS = S // KSP         # 4
    if scale is None:
        scale = DH ** -0.5

    nc = bacc.Bacc(target_bir_lowering=False, debug=False)

    xT_d = nc.dram_tensor("xT", [D, S], BF16, kind="ExternalInput").ap()
    Wq_d = nc.dram_tensor("Wq", [ND, P, ND, P], BF16, kind="ExternalInput").ap()
    Wk_d = nc.dram_tensor("Wk", [ND, P, ND, P], BF16, kind="ExternalInput").ap()
    Wv_d = nc.dram_tensor("Wv", [ND, P, ND, P], BF16, kind="ExternalInput").ap()
    Wo_d = nc.dram_tensor("Wo", [D, D], BF16, kind="ExternalInput").ap()
    bq_d = nc.dram_tensor("bq", [D], F32, kind="ExternalInput").ap()
    bk_d = nc.dram_tensor("bk", [D], F32, kind="ExternalInput").ap()
    bv_d = nc.dram_tensor("bv", [D], F32, kind="ExternalInput").ap()
    bo_d = nc.dram_tensor("bo", [D], BF16, kind="ExternalInput").ap()
    ones_d = nc.dram_tensor("cst_ones", [P, P], BF16, kind="ExternalInput").ap()
    y = nc.dram_tensor("y", [Sq, D], F32, kind="ExternalOutput").ap()

    with tile.TileContext(nc) as tc, ExitStack() as top:
        top.enter_context(nc.allow_low_precision(
            reason="bf16 activations/weights with fp32 psum accumulation"))
        const = top.enter_context(tc.tile_pool(name="const", bufs=1))
        big = top.enter_context(tc.tile_pool(name="big", bufs=1))
        wkv = top.enter_context(tc.tile_pool(name="wkv", bufs=1))
        exq = top.enter_context(tc.tile_pool(name="exp", bufs=4))
        eps = top.enter_context(tc.tile_pool(name="eps", bufs=2))
        stgp = top.enter_context(tc.tile_pool(name="stgp", bufs=3))
        ystg = top.enter_context(tc.tile_pool(name="ystg", bufs=1))
        scps = top.enter_context(tc.tile_pool(name="scps", bufs=2, space="PSUM"))
        opsum = top.enter_context(tc.tile_pool(name="ops", bufs=2, space="PSUM"))
        pps = top.enter_context(tc.tile_pool(name="pps", bufs=1, space="PSUM"))

        # --- exp table warm-up: tiny activation so the ~2.7us ACT table
        # load runs during the DMA-bound preamble ---
        warm_in = const.tile([1, 8], F32)
        nc.vector.memset(warm_in, 0.0)
        warm_out = const.tile([1, 8], BF16)
        nc.scalar.activation(warm_out, warm_in, AF.Exp)

        ones_t = const.tile([1, DH], BF16)
        nc.vector.memset(ones_t, 1.0)

        # biases (small DMAs on the gpsimd queue)
        bq_sb = const.tile([P, ND], F32)
        nc.gpsimd.dma_start(out=bq_sb, in_=bq_d.rearrange("(j p) -> p j", p=P))
        bk_sb = const.tile([P, ND], F32)
        nc.gpsimd.dma_start(out=bk_sb, in_=bk_d.rearrange("(j p) -> p j", p=P))
        bv_even = const.tile([DH, ND], F32)
        nc.gpsimd.dma_start(
            out=bv_even, in_=bv_d.rearrange("(j q p) -> q p j", p=DH, q=2)[0])
        bv_odd = const.tile([DH, ND], F32)
        nc.gpsimd.dma_start(
            out=bv_odd, in_=bv_d.rearrange("(j q p) -> q p j", p=DH, q=2)[1])
        bo_bc = const.tile([P, D], BF16)
        nc.gpsimd.dma_start(
            out=bo_bc,
            in_=bo_d.unsqueeze(0).partition_broadcast(P).squeeze(1))

        # persistent SBUF
        xT = big.tile([P, ND, S], BF16)
        qTs = big.tile([P, ND, Sq], BF16)
        kp_sb = big.tile([P, NPAIR, S], BF16)
        v_sb = big.tile([P, NS, H * W65], BF16)
        oT = big.tile([P, ND, Sq], BF16)
        Wq_sb = big.tile([P, ND, D], BF16)
        Wv_sb = big.tile([P, ND, D], BF16)
        # Wk and Wo share one buffer: Wk's last use (kT pair 7) precedes
        # Wo's first use (output projection in the second q-half pass)
        Wk_sb = wkv.tile([P, ND, D], BF16, tag="w", name="Wk_sb")
        v3 = v_sb.rearrange("p i (h w) -> p i h w", w=W65)

        xT_r = xT_d.rearrange("(j p) s -> p j s", p=P)

        # --- priority-ordered big DMAs; weights for K/V go on the gpsimd
        # queue so they don't serialize behind xT on the sync queue ---
        nc.sync.dma_start(out=xT[:, :, 0:QSP], in_=xT_r[:, :, 0:QSP])
        nc.gpsimd.dma_start(out=Wk_sb[:, :, 0:P], in_=Wk_d[0])
        nc.sync.dma_start(out=Wq_sb[:, :, 0:P], in_=Wq_d[0])
        nc.gpsimd.dma_start(out=Wv_sb[:, :, 0:P], in_=Wv_d[0])
        nc.gpsimd.dma_start(out=Wv_sb[:, :, P:2 * P], in_=Wv_d[1])
        nc.sync.dma_start(out=xT[:, :, QSP:Sq], in_=xT_r[:, :, QSP:Sq])
        nc.gpsimd.dma_start(out=Wk_sb[:, :, P:2 * P], in_=Wk_d[1])
        nc.sync.dma_start(out=Wq_sb[:, :, P:2 * P], in_=Wq_d[1])
        nc.sync.dma_start(out=xT[:, :, Sq:S], in_=xT_r[:, :, Sq:S])
        for s in range(2, ND):
            nc.gpsimd.dma_start(out=Wk_sb[:, :, s * P:(s + 1) * P], in_=Wk_d[s])
            nc.gpsimd.dma_start(out=Wv_sb[:, :, s * P:(s + 1) * P], in_=Wv_d[s])
            nc.sync.dma_start(out=Wq_sb[:, :, s * P:(s + 1) * P], in_=Wq_d[s])

        # --- projection helpers ---
        def qT_proj(p, sp):
            ps = pps.tile([P, QSP], F32, tag="pp", name=f"qps_{p}_{sp}")
            for j in range(ND):
                nc.tensor.matmul(
                    ps,
                    Wq_sb[:, j, p * P:(p + 1) * P],
                    xT[:, j, sp * QSP:(sp + 1) * QSP],
                    start=(j == 0), stop=(j == ND - 1),
                )
            nc.vector.tensor_scalar_add(
                qTs[:, p, sp * QSP:(sp + 1) * QSP], ps, bq_sb[:, p:p + 1])

        def kT_proj(s, span):
            ps = pps.tile([P, KSP], F32, tag="pp", name=f"kps_{s}_{span}")
            for j in range(ND):
                nc.tensor.matmul(
                    ps,
                    Wk_sb[:, j, s * P:(s + 1) * P],
                    xT[:, j, span * KSP:(span + 1) * KSP],
                    start=(j == 0), stop=(j == ND - 1),
                )
            nc.vector.tensor_scalar_add(
                kp_sb[:, s, span * KSP:(span + 1) * KSP], ps, bk_sb[:, s:s + 1])

        def v_proj(i, q):
            # quarter q covers heads 4q..4q+4 (v columns 256*q..256*(q+1))
            if q == 0:
                nc.sync.dma_start(out=v3[:, i, :, DH:DH + 1],
                                  in_=ones_d[:, 0:H].unsqueeze(2))
            ps = pps.tile([P, 2 * P], F32, tag="pp", name=f"vps_{i}_{q}")
            for j in range(ND):
                nc.tensor.matmul(
                    ps,
                    xT[:, j, i * P:(i + 1) * P],
                    Wv_sb[:, j, q * 2 * P:(q + 1) * 2 * P],
                    start=(j == 0), stop=(j == ND - 1),
                )
            nc.vector.tensor_copy(
                v3[:, i, q * 4:(q + 1) * 4, 0:DH],
                ps.rearrange("p (h w) -> p h w", w=DH),
            )

        def wo_dma():
            # reuses Wk's buffer; allocation waits for kT pair 7 to finish
            Wo_sb = wkv.tile([P, ND, D], BF16, tag="w", name="Wo_sb")
            nc.sync.dma_start(
                out=Wo_sb, in_=Wo_d.rearrange("(j p) c -> p j c", p=P))
            return Wo_sb

        wo_holder = []

        def out_proj(sc_i, span):
            Wo_sb = wo_holder[0]
            ps = pps.tile([P, QSP], F32, tag="pp", name=f"yps_{sc_i}_{span}")
            for j in range(ND):
                nc.tensor.matmul(
                    ps,
                    oT[:, j, sc_i * P:(sc_i + 1) * P],
                    Wo_sb[:, j, span * QSP:(span + 1) * QSP],
                    start=(j == 0), stop=(j == ND - 1),
                )
            ysb = ystg.tile([P, QSP], F32, tag="ysb")
            nc.vector.tensor_add(ysb, ps, bo_bc[:, span * QSP:(span + 1) * QSP])
            nc.sync.dma_start(
                out=y[sc_i * P:(sc_i + 1) * P, span * QSP:(span + 1) * QSP],
                in_=ysb,
            )

        # --- deadline-ordered filler queue ---
        work = deque()

        def add(key, cost, fn):
            work.append((key, cost, fn))

        # v quarters B/C/D interleave ahead of their first consumer block;
        # kT/qT for pair p+1 are force-pumped inside block p.
        for i in range(1, NS):
            add(f"vA{i}", 0.95, (lambda i=i: v_proj(i, 0)))
        add("kT0s2", 1.85, lambda: kT_proj(0, 2))
        add("kT0s3", 1.85, lambda: kT_proj(0, 3))
        for s_ in range(NKS):
            add(f"kT1s{s_}", 1.85, (lambda s=s_: kT_proj(1, s)))
        add("qT1s0", 1.85, lambda: qT_proj(1, 0))
        for i in range(NS):
            add(f"vB{i}", 0.95, (lambda i=i: v_proj(i, 1)))
        for s_ in range(NKS):
            add(f"kT2s{s_}", 1.85, (lambda s=s_: kT_proj(2, s)))
        add("qT2s0", 1.85, lambda: qT_proj(2, 0))
        for s_ in range(NKS):
            add(f"kT3s{s_}", 1.85, (lambda s=s_: kT_proj(3, s)))
        add("qT3s0", 1.85, lambda: qT_proj(3, 0))
        for i in range(NS):
            add(f"vC{i}", 0.95, (lambda i=i: v_proj(i, 2)))
        for s_ in range(NKS):
            add(f"kT4s{s_}", 1.85, (lambda s=s_: kT_proj(4, s)))
        add("qT4s0", 1.85, lambda: qT_proj(4, 0))
        for s_ in range(NKS):
            add(f"kT5s{s_}", 1.85, (lambda s=s_: kT_proj(5, s)))
        add("qT5s0", 1.85, lambda: qT_proj(5, 0))
        for i in range(NS):
            add(f"vD{i}", 0.95, (lambda i=i: v_proj(i, 3)))
        for s_ in range(NKS):
            add(f"kT6s{s_}", 1.85, (lambda s=s_: kT_proj(6, s)))
        add("qT6s0", 1.85, lambda: qT_proj(6, 0))
        for s_ in range(NKS):
            add(f"kT7s{s_}", 1.85, (lambda s=s_: kT_proj(7, s)))
        add("qT7s0", 1.85, lambda: qT_proj(7, 0))
        add("WoDMA", 0.0, lambda: wo_holder.append(wo_dma()))
        for p_ in range(NPAIR):
            add(f"qT{p_}s1", 1.85, (lambda p=p_: qT_proj(p, 1)))
        for sc_i in range(4):
            for span in range(2):
                add(f"op0_{sc_i}_{span}", 1.85,
                    (lambda a=sc_i, b=span: out_proj(a, b)))

        emitted = set()

        def pump_one():
            key, cost, fn = work.popleft()
            fn()
            emitted.add(key)
            return cost

        def pump_until(key):
            if key in emitted:
                return
            avail = {k for k, _, _ in work}
            assert key in avail, f"missing work key {key}"
            while key not in emitted:
                pump_one()

        carry = [0.0]

        def pump_credit(credit):
            carry[0] += credit
            while work and carry[0] >= work[0][1]:
                carry[0] -= pump_one()

        # --- preamble projections ---
        qT_proj(0, 0)
        kT_proj(0, 0)
        kT_proj(0, 1)
        v_proj(0, 0)

        # --- attention: q-span outer, head pair inner ---
        for sp in range(NQS):
            for p in range(NPAIR):
                qsl = slice(sp * QSP, (sp + 1) * QSP)
                if not (sp == 0 and p == 0):
                    pump_until(f"qT{p}s{sp}")
                if sp == 0 and p >= 1:
                    pump_until(f"kT{p}s3")
                o_even = opsum.tile([W65, QSP], F32, tag="op")
                o_odd = opsum.tile([W65, QSP], F32, tag="op")
                for kt in range(NS):
                    if sp == 0:
                        # JIT: spread the v quarter this (or next) block needs
                        # and next pair's kT/qT through the iterations
                        if p % 2 == 0 and kt >= 1:
                            pump_until(f"v{QLET[p // 2]}{kt}")
                        if p == 0 and kt == 7:
                            pump_until("kT0s3")
                        if p < NPAIR - 1 and kt in (3, 7, 11):
                            pump_until(f"kT{p + 1}s{kt // 4}")
                        if p < NPAIR - 1 and kt == 14:
                            pump_until(f"qT{p + 1}s0")
                    sc = scps.tile([P, 2 * QSP], F32, tag="sc")
                    nc.tensor.matmul(
                        sc[:, 0:QSP],
                        kp_sb[0:DH, p, kt * P:(kt + 1) * P],
                        qTs[0:DH, p, qsl],
                        start=True, stop=True,
                    )
                    nc.tensor.matmul(
                        sc[:, QSP:2 * QSP],
                        kp_sb[DH:P, p, kt * P:(kt + 1) * P],
                        qTs[DH:P, p, qsl],
                        start=True, stop=True,
                    )
                    ex = exq.tile([P, 2 * QSP], BF16, tag="ex")
                    nc.scalar.activation(ex, sc, AF.Exp, scale=float(scale))
                    nc.tensor.matmul(
                        o_even,
                        v3[:, kt, 2 * p, :],
                        ex[:, 0:QSP],
                        start=(kt == 0), stop=(kt == NS - 1),
                    )
                    nc.tensor.matmul(
                        o_odd,
                        v3[:, kt, 2 * p + 1, :],
                        ex[:, QSP:2 * QSP],
                        start=(kt == 0), stop=(kt == NS - 1),
                    )
                    pump_credit(0.55 if sp == 0 else 0.6)

                # epilogue: drain PSUM to SBUF early, then normalize + bias
                den_e = eps.tile([1, QSP], F32, tag="den")
                nc.vector.tensor_copy(den_e, o_even[DH:W65, :])
                stg_e = stgp.tile([DH, QSP], F32, tag="stg")
                nc.vector.tensor_copy(stg_e, o_even[0:DH, :])
                den_o = eps.tile([1, QSP], F32, tag="den")
                nc.vector.tensor_copy(den_o, o_odd[DH:W65, :])
                stg_o = stgp.tile([DH, QSP], F32, tag="stg")
                nc.vector.tensor_copy(stg_o, o_odd[0:DH, :])
                for par, stg, den in ((0, stg_e, den_e), (1, stg_o, den_o)):
                    rc0 = eps.tile([1, QSP], F32, tag="rc0")
                    nc.vector.reciprocal_approx_fast(rc0, den)
                    rc0b = eps.tile([1, QSP], BF16, tag="rc0b")
                    nc.vector.tensor_copy(rc0b, rc0)
                    rb_ps = pps.tile([DH, QSP], F32, tag="rb")
                    nc.tensor.matmul(rb_ps, ones_t, rc0b, start=True, stop=True)
                    bv_sb = bv_even if par == 0 else bv_odd
                    if par == 0:
                        dst = oT[0:DH, p, qsl]
                        nc.vector.tensor_mul(dst, stg, rb_ps)
                        nc.vector.tensor_scalar_add(dst, dst, bv_sb[:, p:p + 1])
                    else:
                        on = eps.tile([DH, QSP], BF16, tag="on")
                        nc.vector.tensor_mul(on, stg, rb_ps)
                        nc.vector.tensor_scalar_add(on, on, bv_sb[:, p:p + 1])
                        nc.sync.dma_start(out=oT[DH:P, p, qsl], in_=on)

        # drain any remaining queued work
        while work:
            pump_one()

        # --- output projection for the second q-half ---
        for sc_i in range(4, Sq // P):
            for span in range(2):
                out_proj(sc_i, span)

    nc.compile()
    return nc


_NC = None


def _get_nc():
    global _NC
    if _NC is None:
        _NC = build_mha_nc(S=S_FULL, Sq=S_FULL // 2, D=D_FULL, H=H_FULL)
    return _NC


def _chunk_w(w, nblk=8):
    # [D, D] -> [nblk, 128, 8, 128]: block pb -> [p, j, c]
    D = w.shape[0]
    t = w.reshape(D // P, P, nblk, P)             # [j, p, pb, c]
    return np.ascontiguousarray(t.transpose(2, 1, 0, 3))


def shard_inputs(inputs):
    x = np.asarray(inputs["x"], dtype=np.float32)
    shared = {
        "Wq": _chunk_w(np.asarray(inputs["Wq"], np.float32).astype(NPBF16)),
        "Wk": _chunk_w(np.asarray(inputs["Wk"], np.float32).astype(NPBF16)),
        "Wv": _chunk_w(np.asarray(inputs["Wv"], np.float32).astype(NPBF16)),
        "Wo": np.ascontiguousarray(
            np.asarray(inputs["Wo"], np.float32).astype(NPBF16)),
        "bq": np.ascontiguousarray(np.asarray(inputs["bq"], np.float32)),
        "bk": np.ascontiguousarray(np.asarray(inputs["bk"], np.float32)),
        "bv": np.ascontiguousarray(np.asarray(inputs["bv"], np.float32)),
        "bo": np.ascontiguousarray(
            np.asarray(inputs["bo"], np.float32).astype(NPBF16)),
        "cst_ones": np.ones((P, P), dtype=NPBF16),
    }
    half = S_FULL // 2
    maps = []
    for c in range(N_CORES):
        b, role = divmod(c, 2)
        xb = x[b]
        xp = np.concatenate([xb[role * half:(role + 1) * half],
                             xb[(1 - role) * half:(2 - role) * half]], axis=0)
        m = dict(shared)
        m["xT"] = np.ascontiguousarray(xp.T.astype(NPBF16))
        maps.append(m)
    return maps


def run(inputs, trace=False):
    nc = _get_nc()
    maps = shard_inputs(inputs)
    res = run_bass_kernel_spmd(nc, maps, list(range(N_CORES)), trace=trace)
    half = S_FULL // 2
    y = np.empty((B_FULL, S_FULL, D_FULL), dtype=np.float32)
    for c in range(N_CORES):
        b, role = divmod(c, 2)
        y[b, role * half:(role + 1) * half] = res.results[c]["y"]
    return y, res


def kernel(**inputs):
    y, _ = run(inputs, trace=False)
    return y


# revision 15
# speedup vs baseline: 1.0658x; 1.0658x over previous
"""Trainium2 Bass kernel for nn_MultiHeadAttention (B=4, S=2048, D=1024, H=16).

Sharding: 8 cores = 4 batches x 2 query-halves. Each core computes full K/V
projections for its batch (keys permuted so the core's own half comes first),
attention for its 1024 queries over all 2048 keys, and the output projection
for its query half. No collectives.

Per-core dataflow (bf16 matmuls, fp32 PSUM accumulation):
  xT arrives pre-transposed from the host ([D, S] bf16), so there is no
  on-device transpose phase. Attention runs ACT(exp)-paced (head-pair outer,
  q-span inner): per block, 16 key chunks of row-paired score matmuls ->
  exp -> ones-augmented AV matmuls. All projection work (qT/kT/v/output) is
  drip-fed just-in-time into the PE's slack via a deadline-ordered work
  queue, so the scalar engine starts exp'ing within ~12us of launch. Softmax
  denominator rides in the augmented V column; the epilogue drains PSUM to
  SBUF immediately (to unblock the next block's AV accumulation), then does
  reciprocal + K=1 broadcast matmul + bias.
"""

import numpy as np
import ml_dtypes
from collections import deque
from contextlib import ExitStack

import concourse.bass as bass
from concourse import bacc
import concourse.mybir as mybir
import concourse.tile as tile
from concourse.bass_utils import run_bass_kernel_spmd

F32 = mybir.dt.float32
BF16 = mybir.dt.bfloat16
AF = mybir.ActivationFunctionType
NPBF16 = ml_dtypes.bfloat16

P = 128

N_CORES = 8
B_FULL, S_FULL, D_FULL = 4, 2048, 1024
H_FULL, DH = 16, 64
W65 = DH + 1


def build_mha_nc(S=2048, Sq=1024, D=1024, H=16, scale=None):
    assert D % P == 0 and S % P == 0 and Sq % P == 0 and H % 2 == 0
    ND = D // P            # d-chunks (also number of head pairs)
    NS = S // P            # key chunks
    NPAIR = H // 2         # 8 head pairs
    QSP = 512              # q span per attention block
    NQS = Sq // QSP        # 2
    KSP = 512              # kT projection span
    NKS = S // KSP         # 4
    if scale is None:
        scale = DH ** -0.5

    nc = bacc.Bacc(target_bir_lowering=False, debug=False)

    xT_d = nc.dram_tensor("xT", [D, S], BF16, kind="ExternalInput").ap()
    Wq_d = nc.dram_tensor("Wq", [ND, P, ND, P], BF16, kind="ExternalInput").ap()
    Wk_d = nc.dram_tensor("Wk", [ND, P, ND, P], BF16, kind="ExternalInput").ap()
    Wv_d = nc.dram_tensor("Wv", [ND, P, ND, P], BF16, kind="ExternalInput").ap()
    Wo_d = nc.dram_tensor("Wo", [D, D], BF16, kind="ExternalInput").ap()
    bq_d = nc.dram_tensor("bq", [D], F32, kind="ExternalInput").ap()
    bk_d = nc.dram_tensor("bk", [D], F32, kind="ExternalInput").ap()
    bv_d = nc.dram_tensor("bv", [D], F32, kind="ExternalInput").ap()
    bo_d = nc.dram_tensor("bo", [D], BF16, kind="ExternalInput").ap()
    y = nc.dram_tensor("y", [Sq, D], F32, kind="ExternalOutput").ap()

    with tile.TileContext(nc) as tc, ExitStack() as top:
        top.enter_context(nc.allow_low_precision(
            reason="bf16 activations/weights with fp32 psum accumulation"))
        const = top.enter_context(tc.tile_pool(name="const", bufs=1))
        big = top.enter_context(tc.tile_pool(name="big", bufs=1))
        wkv = top.enter_context(tc.tile_pool(name="wkv", bufs=1))
        exq = top.enter_context(tc.tile_pool(name="exp", bufs=4))
        eps = top.enter_context(tc.tile_pool(name="eps", bufs=2))
        stgp = top.enter_context(tc.tile_pool(name="stgp", bufs=3))
        ystg = top.enter_context(tc.tile_pool(name="ystg", bufs=1))
        scps = top.enter_context(tc.tile_pool(name="scps", bufs=2, space="PSUM"))
        opsum = top.enter_context(tc.tile_pool(name="ops", bufs=2, space="PSUM"))
        pps = top.enter_context(tc.tile_pool(name="pps", bufs=1, space="PSUM"))

        # --- exp table warm-up: tiny activation so the ~2.7us ACT table
        # load runs during the DMA-bound preamble ---
        warm_in = const.tile([1, 8], F32)
        nc.vector.memset(warm_in, 0.0)
        warm_out = const.tile([1, 8], BF16)
        nc.scalar.activation(warm_out, warm_in, AF.Exp)

        ones_t = const.tile([1, DH], BF16)
        nc.vector.memset(ones_t, 1.0)

        # biases (small DMAs on the gpsimd queue)
        bq_sb = const.tile([P, ND], F32)
        nc.gpsimd.dma_start(out=bq_sb, in_=bq_d.rearrange("(j p) -> p j", p=P))
        bk_sb = const.tile([P, ND], F32)
        nc.gpsimd.dma_start(out=bk_sb, in_=bk_d.rearrange("(j p) -> p j", p=P))
        bv_even = const.tile([DH, ND], F32)
        nc.gpsimd.dma_start(
            out=bv_even, in_=bv_d.rearrange("(j q p) -> q p j", p=DH, q=2)[0])
        bv_odd = const.tile([DH, ND], F32)
        nc.gpsimd.dma_start(
            out=bv_odd, in_=bv_d.rearrange("(j q p) -> q p j", p=DH, q=2)[1])
        bo_bc = const.tile([P, D], BF16)
        nc.gpsimd.dma_start(
            out=bo_bc,
            in_=bo_d.unsqueeze(0).partition_broadcast(P).squeeze(1))

        # persistent SBUF
        xT = big.tile([P, ND, S], BF16)
        qTs = big.tile([P, ND, Sq], BF16)
        kp_sb = big.tile([P, NPAIR, S], BF16)
        v_sb = big.tile([P, NS, H * W65], BF16)
        oT = big.tile([P, ND, Sq], BF16)
        Wq_sb = big.tile([P, ND, D], BF16)
        Wv_sb = big.tile([P, ND, D], BF16)
        # Wk and Wo share one buffer: Wk's last use (kT pair 7) precedes
        # Wo's first use (output projection)
        Wk_sb = wkv.tile([P, ND, D], BF16, tag="w", name="Wk_sb")
        v3 = v_sb.rearrange("p i (h w) -> p i h w", w=W65)

        # softmax-denominator ones column for all chunks/heads, in one op
        nc.vector.memset(v3[:, :, :, DH:DH + 1], 1.0)

        xT_r = xT_d.rearrange("(j p) s -> p j s", p=P)

        # --- priority-ordered big DMAs; K/V weights go on the gpsimd queue
        # so they don't serialize behind xT on the sync queue ---
        nc.sync.dma_start(out=xT[:, :, 0:QSP], in_=xT_r[:, :, 0:QSP])
        nc.gpsimd.dma_start(out=Wk_sb[:, :, 0:P], in_=Wk_d[0])
        nc.sync.dma_start(out=Wq_sb[:, :, 0:P], in_=Wq_d[0])
        for s in range(4):
            nc.gpsimd.dma_start(out=Wv_sb[:, :, s * P:(s + 1) * P], in_=Wv_d[s])
        nc.sync.dma_start(out=xT[:, :, QSP:Sq], in_=xT_r[:, :, QSP:Sq])
        nc.gpsimd.dma_start(out=Wk_sb[:, :, P:2 * P], in_=Wk_d[1])
        nc.sync.dma_start(out=Wq_sb[:, :, P:2 * P], in_=Wq_d[1])
        nc.sync.dma_start(out=xT[:, :, Sq:S], in_=xT_r[:, :, Sq:S])
        for s in range(2, ND):
            nc.gpsimd.dma_start(out=Wk_sb[:, :, s * P:(s + 1) * P], in_=Wk_d[s])
            nc.sync.dma_start(out=Wq_sb[:, :, s * P:(s + 1) * P], in_=Wq_d[s])
        for s in range(4, ND):
            nc.gpsimd.dma_start(out=Wv_sb[:, :, s * P:(s + 1) * P], in_=Wv_d[s])

        # --- projection helpers ---
        def qT_proj(p, sp):
            ps = pps.tile([P, QSP], F32, tag="pp", name=f"qps_{p}_{sp}")
            for j in range(ND):
                nc.tensor.matmul(
                    ps,
                    Wq_sb[:, j, p * P:(p + 1) * P],
                    xT[:, j, sp * QSP:(sp + 1) * QSP],
                    start=(j == 0), stop=(j == ND - 1),
                )
            nc.vector.tensor_scalar_add(
                qTs[:, p, sp * QSP:(sp + 1) * QSP], ps, bq_sb[:, p:p + 1])

        def kT_proj(s, span):
            ps = pps.tile([P, KSP], F32, tag="pp", name=f"kps_{s}_{span}")
            for j in range(ND):
                nc.tensor.matmul(
                    ps,
                    Wk_sb[:, j, s * P:(s + 1) * P],
                    xT[:, j, span * KSP:(span + 1) * KSP],
                    start=(j == 0), stop=(j == ND - 1),
                )
            nc.vector.tensor_scalar_add(
                kp_sb[:, s, span * KSP:(span + 1) * KSP], ps, bk_sb[:, s:s + 1])

        def v_proj(i, half):
            # half covers heads 8*half..8*half+8 (v columns 512*half..)
            ps = pps.tile([P, QSP], F32, tag="pp", name=f"vps_{i}_{half}")
            for j in range(ND):
                nc.tensor.matmul(
                    ps,
                    xT[:, j, i * P:(i + 1) * P],
                    Wv_sb[:, j, half * QSP:(half + 1) * QSP],
                    start=(j == 0), stop=(j == ND - 1),
                )
            nc.vector.tensor_copy(
                v3[:, i, half * 8:(half + 1) * 8, 0:DH],
                ps.rearrange("p (h w) -> p h w", w=DH),
            )

        def wo_dma():
            # reuses Wk's buffer; allocation waits for kT pair 7 to finish
            Wo_sb = wkv.tile([P, ND, D], BF16, tag="w", name="Wo_sb")
            nc.sync.dma_start(
                out=Wo_sb, in_=Wo_d.rearrange("(j p) c -> p j c", p=P))
            return Wo_sb

        wo_holder = []

        def out_proj(sc_i, span):
            Wo_sb = wo_holder[0]
            ps = pps.tile([P, QSP], F32, tag="pp", name=f"yps_{sc_i}_{span}")
            for j in range(ND):
                nc.tensor.matmul(
                    ps,
                    oT[:, j, sc_i * P:(sc_i + 1) * P],
                    Wo_sb[:, j, span * QSP:(span + 1) * QSP],
                    start=(j == 0), stop=(j == ND - 1),
                )
            ysb = ystg.tile([P, QSP], F32, tag="ysb")
            nc.vector.tensor_add(ysb, ps, bo_bc[:, span * QSP:(span + 1) * QSP])
            nc.sync.dma_start(
                out=y[sc_i * P:(sc_i + 1) * P, span * QSP:(span + 1) * QSP],
                in_=ysb,
            )

        # --- deadline-ordered filler queue ---
        work = deque()

        def add(key, cost, fn):
            work.append((key, cost, fn))

        for i in range(1, NS):
            add(f"vA{i}", 1.85, (lambda i=i: v_proj(i, 0)))
        add("kT0s2", 1.85, lambda: kT_proj(0, 2))
        add("kT0s3", 1.85, lambda: kT_proj(0, 3))
        add("qT0s1", 1.85, lambda: qT_proj(0, 1))
        for s_ in range(NKS):
            add(f"kT1s{s_}", 1.85, (lambda s=s_: kT_proj(1, s)))
        add("qT1s0", 1.85, lambda: qT_proj(1, 0))
        add("qT1s1", 1.85, lambda: qT_proj(1, 1))
        for i in range(8):
            add(f"vB{i}", 1.85, (lambda i=i: v_proj(i, 1)))
        for s_ in range(NKS):
            add(f"kT2s{s_}", 1.85, (lambda s=s_: kT_proj(2, s)))
        add("qT2s0", 1.85, lambda: qT_proj(2, 0))
        add("qT2s1", 1.85, lambda: qT_proj(2, 1))
        for i in range(8, NS):
            add(f"vB{i}", 1.85, (lambda i=i: v_proj(i, 1)))
        for p_ in range(3, NPAIR):
            for s_ in range(NKS):
                add(f"kT{p_}s{s_}", 1.85, (lambda s=s_, q=p_: kT_proj(q, s)))
            add(f"qT{p_}s0", 1.85, (lambda q=p_: qT_proj(q, 0)))
            add(f"qT{p_}s1", 1.85, (lambda q=p_: qT_proj(q, 1)))
        add("WoDMA", 0.0, lambda: wo_holder.append(wo_dma()))
        for sc_i in range(4):
            for span in range(2):
                add(f"op0_{sc_i}_{span}", 1.85,
                    (lambda a=sc_i, b=span: out_proj(a, b)))

        emitted = set()
        op0_ok = [False]

        def pump_one():
            key, cost, fn = work.popleft()
            fn()
            emitted.add(key)
            return cost

        def pump_until(key):
            if key in emitted:
                return
            avail = {k for k, _, _ in work}
            assert key in avail, f"missing work key {key}"
            while key not in emitted:
                pump_one()

        carry = [0.0]

        def pump_credit(credit):
            carry[0] += credit
            while work and carry[0] >= work[0][1]:
                if work[0][0].startswith("op0") and not op0_ok[0]:
                    return
                carry[0] -= pump_one()

        # --- preamble projections ---
        qT_proj(0, 0)
        kT_proj(0, 0)
        kT_proj(0, 1)
        v_proj(0, 0)

        # --- attention: head pair outer, q-span inner ---
        for p in range(NPAIR):
            for sp in range(NQS):
                qsl = slice(sp * QSP, (sp + 1) * QSP)
                if not (sp == 0 and p == 0):
                    pump_until(f"qT{p}s{sp}")
                if sp == 0 and p >= 1:
                    pump_until(f"kT{p}s3")
                if sp == 0 and p >= 4:
                    pump_until(f"vB{NS - 1}")
                if sp == 1 and p == NPAIR - 1:
                    op0_ok[0] = True
                o_even = opsum.tile([W65, QSP], F32, tag="op")
                o_odd = opsum.tile([W65, QSP], F32, tag="op")
                for kt in range(NS):
                    if p == 0 and sp == 0:
                        if kt >= 1:
                            pump_until(f"vA{kt}")
                        if kt == 7:
                            pump_until("kT0s2")
                        if kt == 11:
                            pump_until("kT0s3")
                    if sp == 0 and p < NPAIR - 1 and kt in (3, 7, 11):
                        pump_until(f"kT{p + 1}s{kt // 4}")
                    if sp == 0 and kt == 13:
                        pump_until(f"qT{p}s1")
                    if sp == 0 and p < NPAIR - 1 and kt == 15:
                        pump_until(f"qT{p + 1}s0")
                    sc = scps.tile([P, 2 * QSP], F32, tag="sc")
                    nc.tensor.matmul(
                        sc[:, 0:QSP],
                        kp_sb[0:DH, p, kt * P:(kt + 1) * P],
                        qTs[0:DH, p, qsl],
                        start=True, stop=True,
                    )
                    nc.tensor.matmul(
                        sc[:, QSP:2 * QSP],
                        kp_sb[DH:P, p, kt * P:(kt + 1) * P],
                        qTs[DH:P, p, qsl],
                        start=True, stop=True,
                    )
                    ex = exq.tile([P, 2 * QSP], BF16, tag="ex")
                    nc.scalar.activation(ex, sc, AF.Exp, scale=float(scale))
                    nc.tensor.matmul(
                        o_even,
                        v3[:, kt, 2 * p, :],
                        ex[:, 0:QSP],
                        start=(kt == 0), stop=(kt == NS - 1),
                    )
                    nc.tensor.matmul(
                        o_odd,
                        v3[:, kt, 2 * p + 1, :],
                        ex[:, QSP:2 * QSP],
                        start=(kt == 0), stop=(kt == NS - 1),
                    )
                    pump_credit(0.55)

                # epilogue: drain PSUM to SBUF early, then normalize + bias
                den_e = eps.tile([1, QSP], F32, tag="den")
                nc.vector.tensor_copy(den_e, o_even[DH:W65, :])
                stg_e = stgp.tile([DH, QSP], F32, tag="stg")
                nc.vector.tensor_copy(stg_e, o_even[0:DH, :])
                den_o = eps.tile([1, QSP], F32, tag="den")
                nc.vector.tensor_copy(den_o, o_odd[DH:W65, :])
                stg_o = stgp.tile([DH, QSP], F32, tag="stg")
                nc.vector.tensor_copy(stg_o, o_odd[0:DH, :])
                for par, stg, den in ((0, stg_e, den_e), (1, stg_o, den_o)):
                    rc0 = eps.tile([1, QSP], F32, tag="rc0")
                    nc.vector.reciprocal_approx_fast(rc0, den)
                    rc0b = eps.tile([1, QSP], BF16, tag="rc0b")
                    nc.vector.tensor_copy(rc0b, rc0)
                    rb_ps = pps.tile([DH, QSP], F32, tag="rb")
                    nc.tensor.matmul(rb_ps, ones_t, rc0b, start=True, stop=True)
                    bv_sb = bv_even if par == 0 else bv_odd
                    if par == 0:
                        dst = oT[0:DH, p, qsl]
                        nc.vector.tensor_mul(dst, stg, rb_ps)
                        nc.vector.tensor_scalar_add(dst, dst, bv_sb[:, p:p + 1])
                    else:
                        on = eps.tile([DH, QSP], BF16, tag="on")
                        nc.vector.tensor_mul(on, stg, rb_ps)
                        nc.vector.tensor_scalar_add(on, on, bv_sb[:, p:p + 1])
                        nc.sync.dma_start(out=oT[DH:P, p, qsl], in_=on)

        # drain any remaining queued work
        while work:
            pump_one()

        # --- output projection for the second q-half ---
        for sc_i in range(4, Sq // P):
            for span in range(2):
                out_proj(sc_i, span)

    nc.compile()
    return nc


_NC = None


def _get_nc():
    global _NC
    if _NC is None:
        _NC = build_mha_nc(S=S_FULL, Sq=S_FULL // 2, D=D_FULL, H=H_FULL)
    return _NC


def _chunk_w(w, nblk=8):
    # [D, D] -> [nblk, 128, 8, 128]: block pb -> [p, j, c]
    D = w.shape[0]
    t = w.reshape(D // P, P, nblk, P)             # [j, p, pb, c]
    return np.ascontiguousarray(t.transpose(2, 1, 0, 3))


def shard_inputs(inputs):
    x = np.asarray(inputs["x"], dtype=np.float32)
    shared = {
        "Wq": _chunk_w(np.asarray(inputs["Wq"], np.float32).astype(NPBF16)),
        "Wk": _chunk_w(np.asarray(inputs["Wk"], np.float32).astype(NPBF16)),
        "Wv": _chunk_w(np.asarray(inputs["Wv"], np.float32).astype(NPBF16)),
        "Wo": np.ascontiguousarray(
            np.asarray(inputs["Wo"], np.float32).astype(NPBF16)),
        "bq": np.ascontiguousarray(np.asarray(inputs["bq"], np.float32)),
        "bk": np.ascontiguousarray(np.asarray(inputs["bk"], np.float32)),
        "bv": np.ascontiguousarray(np.asarray(inputs["bv"], np.float32)),
        "bo": np.ascontiguousarray(
            np.asarray(inputs["bo"], np.float32).astype(NPBF16)),
    }
    half = S_FULL // 2
    maps = []
    for c in range(N_CORES):
        b, role = divmod(c, 2)
        xb = x[b]
        xp = np.concatenate([xb[role * half:(role + 1) * half],
                             xb[(1 - role) * half:(2 - role) * half]], axis=0)
        m = dict(shared)
        m["xT"] = np.ascontiguousarray(xp.T.astype(NPBF16))
        maps.append(m)
    return maps


def run(inputs, trace=False):
    nc = _get_nc()
    maps = shard_inputs(inputs)
    res = run_bass_kernel_spmd(nc, maps, list(range(N_CORES)), trace=trace)
    half = S_FULL // 2
    y = np.empty((B_FULL, S_FULL, D_FULL), dtype=np.float32)
    for c in range(N_CORES):
        b, role = divmod(c, 2)
        y[b, role * half:(role + 1) * half] = res.results[c]["y"]
    return y, res


def kernel(**inputs):
    y, _ = run(inputs, trace=False)
    return y


# revision 23
# speedup vs baseline: 1.1130x; 1.0442x over previous
"""Trainium2 Bass kernel for nn_MultiHeadAttention (B=4, S=2048, D=1024, H=16).

Sharding: 8 cores = 4 batches x 2 query-halves. Each core computes full K/V
projections for its batch (keys permuted so the core's own half comes first),
attention for its 1024 queries over all 2048 keys, and the output projection
for its query half. No collectives.

Per-core dataflow (bf16 matmuls, fp32 PSUM accumulation):
  xT arrives pre-transposed from the host ([D, S] bf16), so there is no
  on-device transpose phase. Attention runs ACT(exp)-paced (head-pair outer,
  q-span inner): per block, 16 key chunks of row-paired score matmuls ->
  exp -> ones-augmented AV matmuls. All projection work (qT/kT/v/output) is
  drip-fed just-in-time into the PE's slack via a deadline-ordered work
  queue, so the scalar engine starts exp'ing within ~12us of launch. Softmax
  denominator rides in the augmented V column; the epilogue drains PSUM to
  SBUF immediately (to unblock the next block's AV accumulation), then does
  reciprocal + K=1 broadcast matmul + bias.
"""

import numpy as np
import ml_dtypes
from collections import deque
from contextlib import ExitStack

import concourse.bass as bass
from concourse import bacc
import concourse.mybir as mybir
import concourse.tile as tile
from concourse.bass_utils import run_bass_kernel_spmd

F32 = mybir.dt.float32
BF16 = mybir.dt.bfloat16
AF = mybir.ActivationFunctionType
NPBF16 = ml_dtypes.bfloat16

P = 128

N_CORES = 8
B_FULL, S_FULL, D_FULL = 4, 2048, 1024
H_FULL, DH = 16, 64
W65 = DH + 1


def build_mha_nc(S=2048, Sq=1024, D=1024, H=16, scale=None):
    assert D % P == 0 and S % P == 0 and Sq % P == 0 and H % 2 == 0
    ND = D // P            # d-chunks (also number of head pairs)
    NS = S // P            # key chunks
    NPAIR = H // 2         # 8 head pairs
    QSP = 512              # q span per attention block
    NQS = Sq // QSP        # 2
    KSP = 512              # kT projection span
    NKS = S // KSP         # 4
    if scale is None:
        scale = DH ** -0.5

    nc = bacc.Bacc(target_bir_lowering=False, debug=False)

    xT_d = nc.dram_tensor("xT", [D, S], BF16, kind="ExternalInput").ap()
    Wq_d = nc.dram_tensor("Wq", [ND, P, ND, P], BF16, kind="ExternalInput").ap()
    Wk_d = nc.dram_tensor("Wk", [ND, P, ND, P], BF16, kind="ExternalInput").ap()
    Wv_d = nc.dram_tensor("Wv", [ND, P, ND, P], BF16, kind="ExternalInput").ap()
    Wo_d = nc.dram_tensor("Wo", [D, D], BF16, kind="ExternalInput").ap()
    bq_d = nc.dram_tensor("bq", [D], F32, kind="ExternalInput").ap()
    bk_d = nc.dram_tensor("bk", [D], F32, kind="ExternalInput").ap()
    bv_d = nc.dram_tensor("bv", [D], F32, kind="ExternalInput").ap()
    bo_d = nc.dram_tensor("bo", [D], BF16, kind="ExternalInput").ap()
    y = nc.dram_tensor("y", [Sq, D], F32, kind="ExternalOutput").ap()

    with tile.TileContext(nc) as tc, ExitStack() as top:
        top.enter_context(nc.allow_low_precision(
            reason="bf16 activations/weights with fp32 psum accumulation"))
        const = top.enter_context(tc.tile_pool(name="const", bufs=1))
        big = top.enter_context(tc.tile_pool(name="big", bufs=1))
        wkv = top.enter_context(tc.tile_pool(name="wkv", bufs=1))
        exq = top.enter_context(tc.tile_pool(name="exp", bufs=4))
        eps = top.enter_context(tc.tile_pool(name="eps", bufs=2))
        stgp = top.enter_context(tc.tile_pool(name="stgp", bufs=2))
        ystg = top.enter_context(tc.tile_pool(name="ystg", bufs=1))
        scps = top.enter_context(tc.tile_pool(name="scps", bufs=2, space="PSUM"))
        opsum = top.enter_context(tc.tile_pool(name="ops", bufs=2, space="PSUM"))
        pps = top.enter_context(tc.tile_pool(name="pps", bufs=2, space="PSUM"))

        # --- exp table warm-up: tiny activation so the ~2.7us ACT table
        # load runs during the DMA-bound preamble ---
        warm_in = const.tile([1, 8], F32)
        nc.vector.memset(warm_in, 0.0)
        warm_out = const.tile([1, 8], BF16)
        nc.scalar.activation(warm_out, warm_in, AF.Exp)

        # biases (small DMAs on the gpsimd queue)
        bq_sb = const.tile([P, ND], F32)
        nc.gpsimd.dma_start(out=bq_sb, in_=bq_d.rearrange("(j p) -> p j", p=P))
        bk_sb = const.tile([P, ND], F32)
        nc.gpsimd.dma_start(out=bk_sb, in_=bk_d.rearrange("(j p) -> p j", p=P))
        bv_even = const.tile([DH, ND], F32)
        nc.gpsimd.dma_start(
            out=bv_even, in_=bv_d.rearrange("(j q p) -> q p j", p=DH, q=2)[0])
        bv_odd = const.tile([DH, ND], F32)
        nc.gpsimd.dma_start(
            out=bv_odd, in_=bv_d.rearrange("(j q p) -> q p j", p=DH, q=2)[1])
        bo_bc = const.tile([P, D], BF16)
        nc.gpsimd.dma_start(
            out=bo_bc,
            in_=bo_d.unsqueeze(0).partition_broadcast(P).squeeze(1))

        # persistent SBUF
        xT = big.tile([P, ND, S], BF16)
        qTs = big.tile([P, ND, Sq], BF16)
        kp_sb = big.tile([P, NPAIR, S], BF16)
        v_sb = big.tile([P, NS, H * W65], BF16)
        oT = big.tile([P, ND, Sq], BF16)
        Wq_sb = big.tile([P, ND, D], BF16)
        Wv_sb = big.tile([P, ND, D], BF16)
        # Wk and Wo share one buffer: Wk's last use (kT pair 7) precedes
        # Wo's first use (output projection)
        Wk_sb = wkv.tile([P, ND, D], BF16, tag="w", name="Wk_sb")
        v3 = v_sb.rearrange("p i (h w) -> p i h w", w=W65)

        # softmax-denominator ones column for all chunks/heads, in one op
        nc.vector.memset(v3[:, :, :, DH:DH + 1], 1.0)

        xT_r = xT_d.rearrange("(j p) s -> p j s", p=P)

        # --- priority-ordered big DMAs; K/V weights go on the gpsimd queue
        # so they don't serialize behind xT on the sync queue ---
        nc.sync.dma_start(out=xT[:, :, 0:QSP], in_=xT_r[:, :, 0:QSP])
        nc.gpsimd.dma_start(out=Wk_sb[:, :, 0:P], in_=Wk_d[0])
        nc.sync.dma_start(out=Wq_sb[:, :, 0:P], in_=Wq_d[0])
        for s in range(4):
            nc.gpsimd.dma_start(out=Wv_sb[:, :, s * P:(s + 1) * P], in_=Wv_d[s])
        nc.sync.dma_start(out=xT[:, :, QSP:Sq], in_=xT_r[:, :, QSP:Sq])
        nc.gpsimd.dma_start(out=Wk_sb[:, :, P:2 * P], in_=Wk_d[1])
        nc.sync.dma_start(out=Wq_sb[:, :, P:2 * P], in_=Wq_d[1])
        nc.sync.dma_start(out=xT[:, :, Sq:S], in_=xT_r[:, :, Sq:S])
        for s in range(2, ND):
            nc.gpsimd.dma_start(out=Wk_sb[:, :, s * P:(s + 1) * P], in_=Wk_d[s])
            nc.sync.dma_start(out=Wq_sb[:, :, s * P:(s + 1) * P], in_=Wq_d[s])
        for s in range(4, ND):
            nc.gpsimd.dma_start(out=Wv_sb[:, :, s * P:(s + 1) * P], in_=Wv_d[s])

        # --- projection helpers ---
        def qT_proj(p, sp):
            ps = pps.tile([P, QSP], F32, tag="pp", name=f"qps_{p}_{sp}")
            for j in range(ND):
                nc.tensor.matmul(
                    ps,
                    Wq_sb[:, j, p * P:(p + 1) * P],
                    xT[:, j, sp * QSP:(sp + 1) * QSP],
                    start=(j == 0), stop=(j == ND - 1),
                )
            nc.vector.tensor_scalar_add(
                qTs[:, p, sp * QSP:(sp + 1) * QSP], ps, bq_sb[:, p:p + 1])

        def kT_proj(s, span):
            ps = pps.tile([P, KSP], F32, tag="pp", name=f"kps_{s}_{span}")
            for j in range(ND):
                nc.tensor.matmul(
                    ps,
                    Wk_sb[:, j, s * P:(s + 1) * P],
                    xT[:, j, span * KSP:(span + 1) * KSP],
                    start=(j == 0), stop=(j == ND - 1),
                )
            nc.vector.tensor_scalar_add(
                kp_sb[:, s, span * KSP:(span + 1) * KSP], ps, bk_sb[:, s:s + 1])

        def v_proj(i, half):
            # half covers heads 8*half..8*half+8 (v columns 512*half..)
            ps = pps.tile([P, QSP], F32, tag="pp", name=f"vps_{i}_{half}")
            for j in range(ND):
                nc.tensor.matmul(
                    ps,
                    xT[:, j, i * P:(i + 1) * P],
                    Wv_sb[:, j, half * QSP:(half + 1) * QSP],
                    start=(j == 0), stop=(j == ND - 1),
                )
            nc.vector.tensor_copy(
                v3[:, i, half * 8:(half + 1) * 8, 0:DH],
                ps.rearrange("p (h w) -> p h w", w=DH),
            )

        def wo_dma():
            # reuses Wk's buffer; allocation waits for kT pair 7 to finish
            Wo_sb = wkv.tile([P, ND, D], BF16, tag="w", name="Wo_sb")
            nc.sync.dma_start(
                out=Wo_sb, in_=Wo_d.rearrange("(j p) c -> p j c", p=P))
            return Wo_sb

        wo_holder = []

        def out_proj(sc_i, span):
            Wo_sb = wo_holder[0]
            ps = pps.tile([P, QSP], F32, tag="pp", name=f"yps_{sc_i}_{span}")
            for j in range(ND):
                nc.tensor.matmul(
                    ps,
                    oT[:, j, sc_i * P:(sc_i + 1) * P],
                    Wo_sb[:, j, span * QSP:(span + 1) * QSP],
                    start=(j == 0), stop=(j == ND - 1),
                )
            ysb = ystg.tile([P, QSP], F32, tag="ysb")
            nc.vector.tensor_add(ysb, ps, bo_bc[:, span * QSP:(span + 1) * QSP])
            nc.sync.dma_start(
                out=y[sc_i * P:(sc_i + 1) * P, span * QSP:(span + 1) * QSP],
                in_=ysb,
            )

        # --- deadline-ordered filler queue ---
        work = deque()

        def add(key, cost, fn):
            work.append((key, cost, fn))

        for i in range(1, NS):
            add(f"vA{i}", 1.85, (lambda i=i: v_proj(i, 0)))
        add("kT0s2", 1.85, lambda: kT_proj(0, 2))
        add("kT0s3", 1.85, lambda: kT_proj(0, 3))
        add("qT0s1", 1.85, lambda: qT_proj(0, 1))
        for s_ in range(NKS):
            add(f"kT1s{s_}", 1.85, (lambda s=s_: kT_proj(1, s)))
        add("qT1s0", 1.85, lambda: qT_proj(1, 0))
        add("qT1s1", 1.85, lambda: qT_proj(1, 1))
        for i in range(8):
            add(f"vB{i}", 1.85, (lambda i=i: v_proj(i, 1)))
        for s_ in range(NKS):
            add(f"kT2s{s_}", 1.85, (lambda s=s_: kT_proj(2, s)))
        add("qT2s0", 1.85, lambda: qT_proj(2, 0))
        add("qT2s1", 1.85, lambda: qT_proj(2, 1))
        for i in range(8, NS):
            add(f"vB{i}", 1.85, (lambda i=i: v_proj(i, 1)))
        for p_ in range(3, NPAIR):
            for s_ in range(NKS):
                add(f"kT{p_}s{s_}", 1.85, (lambda s=s_, q=p_: kT_proj(q, s)))
            add(f"qT{p_}s0", 1.85, (lambda q=p_: qT_proj(q, 0)))
            add(f"qT{p_}s1", 1.85, (lambda q=p_: qT_proj(q, 1)))
        add("WoDMA", 0.0, lambda: wo_holder.append(wo_dma()))
        for sc_i in range(4):
            for span in range(2):
                add(f"op0_{sc_i}_{span}", 1.85,
                    (lambda a=sc_i, b=span: out_proj(a, b)))

        emitted = set()
        op0_ok = [False]

        def pump_one():
            key, cost, fn = work.popleft()
            fn()
            emitted.add(key)
            return cost

        def pump_until(key):
            if key in emitted:
                return
            avail = {k for k, _, _ in work}
            assert key in avail, f"missing work key {key}"
            while key not in emitted:
                pump_one()

        carry = [0.0]

        def pump_credit(credit):
            carry[0] += credit
            while work and carry[0] >= work[0][1]:
                if work[0][0].startswith("op0") and not op0_ok[0]:
                    return
                carry[0] -= pump_one()

        # --- preamble projections ---
        qT_proj(0, 0)
        kT_proj(0, 0)
        kT_proj(0, 1)
        v_proj(0, 0)

        # --- attention: head pair outer, q-span inner ---
        for p in range(NPAIR):
            for sp in range(NQS):
                qsl = slice(sp * QSP, (sp + 1) * QSP)
                if not (sp == 0 and p == 0):
                    pump_until(f"qT{p}s{sp}")
                if sp == 0 and p >= 1:
                    pump_until(f"kT{p}s3")
                if sp == 0 and p >= 4:
                    pump_until(f"vB{NS - 1}")
                if sp == 0 and p == NPAIR - 1:
                    pump_until("WoDMA")
                if sp == 1 and p == NPAIR - 1:
                    op0_ok[0] = True
                o_even = opsum.tile([W65, QSP], F32, tag="op")
                o_odd = opsum.tile([W65, QSP], F32, tag="op")
                for kt in range(NS):
                    if p == 0 and sp == 0:
                        if kt >= 1:
                            pump_until(f"vA{kt}")
                        if kt == 7:
                            pump_until("kT0s2")
                        if kt == 11:
                            pump_until("kT0s3")
                    if sp == 0 and p < NPAIR - 1 and kt in (3, 7, 11):
                        pump_until(f"kT{p + 1}s{kt // 4}")
                    if sp == 0 and kt == 13:
                        pump_until(f"qT{p}s1")
                    if sp == 0 and p < NPAIR - 1 and kt == 15:
                        pump_until(f"qT{p + 1}s0")
                    sc = scps.tile([P, 2 * QSP], F32, tag="sc")
                    nc.tensor.matmul(
                        sc[:, 0:QSP],
                        kp_sb[0:DH, p, kt * P:(kt + 1) * P],
                        qTs[0:DH, p, qsl],
                        start=True, stop=True,
                    )
                    nc.tensor.matmul(
                        sc[:, QSP:2 * QSP],
                        kp_sb[DH:P, p, kt * P:(kt + 1) * P],
                        qTs[DH:P, p, qsl],
                        start=True, stop=True,
                    )
                    ex = exq.tile([P, 2 * QSP], BF16, tag="ex")
                    nc.scalar.activation(ex, sc, AF.Exp, scale=float(scale))
                    nc.tensor.matmul(
                        o_even,
                        v3[:, kt, 2 * p, :],
                        ex[:, 0:QSP],
                        start=(kt == 0), stop=(kt == NS - 1),
                    )
                    nc.tensor.matmul(
                        o_odd,
                        v3[:, kt, 2 * p + 1, :],
                        ex[:, QSP:2 * QSP],
                        start=(kt == 0), stop=(kt == NS - 1),
                    )
                    pump_credit(1.2 if (p == NPAIR - 1 and sp == 1) else 0.55)

                # epilogue: drain PSUM to SBUF early, then normalize + bias
                den_e = eps.tile([1, QSP], F32, tag="den")
                nc.vector.tensor_copy(den_e, o_even[DH:W65, :])
                stg_e = stgp.tile([DH, QSP], F32, tag="stg")
                nc.vector.tensor_copy(stg_e, o_even[0:DH, :])
                den_o = eps.tile([1, QSP], F32, tag="den")
                nc.vector.tensor_copy(den_o, o_odd[DH:W65, :])
                stg_o = stgp.tile([DH, QSP], F32, tag="stg")
                nc.vector.tensor_copy(stg_o, o_odd[0:DH, :])
                for par, stg, den in ((0, stg_e, den_e), (1, stg_o, den_o)):
                    rc0 = eps.tile([1, QSP], F32, tag="rc0")
                    nc.vector.reciprocal_approx_fast(rc0, den)
                    rc0b = eps.tile([1, QSP], BF16, tag="rc0b")
                    nc.vector.tensor_copy(rc0b, rc0)
                    rb = eps.tile([DH, QSP], BF16, tag="rb")
                    nc.gpsimd.partition_broadcast(rb, rc0b)
                    bv_sb = bv_even if par == 0 else bv_odd
                    if par == 0:
                        dst = oT[0:DH, p, qsl]
                        nc.vector.tensor_mul(dst, stg, rb)
                        nc.vector.tensor_scalar_add(dst, dst, bv_sb[:, p:p + 1])
                    else:
                        on = eps.tile([DH, QSP], BF16, tag="on")
                        nc.vector.tensor_mul(on, stg, rb)
                        nc.vector.tensor_scalar_add(on, on, bv_sb[:, p:p + 1])
                        nc.sync.dma_start(out=oT[DH:P, p, qsl], in_=on)

        # drain any remaining queued work
        while work:
            pump_one()

        # --- output projection for the second q-half ---
        for sc_i in range(4, Sq // P):
            for span in range(2):
                out_proj(sc_i, span)

    nc.compile()
    return nc


_NC = None


def _get_nc():
    global _NC
    if _NC is None:
        _NC = build_mha_nc(S=S_FULL, Sq=S_FULL // 2, D=D_FULL, H=H_FULL)
    return _NC


def _chunk_w(w, nblk=8):
    # [D, D] -> [nblk, 128, 8, 128]: block pb -> [p, j, c]
    D = w.shape[0]
    t = w.reshape(D // P, P, nblk, P)             # [j, p, pb, c]
    return np.ascontiguousarray(t.transpose(2, 1, 0, 3))


def shard_inputs(inputs):
    x = np.asarray(inputs["x"], dtype=np.float32)
    shared = {
        "Wq": _chunk_w(np.asarray(inputs["Wq"], np.float32).astype(NPBF16)),
        "Wk": _chunk_w(np.asarray(inputs["Wk"], np.float32).astype(NPBF16)),
        "Wv": _chunk_w(np.asarray(inputs["Wv"], np.float32).astype(NPBF16)),
        "Wo": np.ascontiguousarray(
            np.asarray(inputs["Wo"], np.float32).astype(NPBF16)),
        "bq": np.ascontiguousarray(np.asarray(inputs["bq"], np.float32)),
        "bk": np.ascontiguousarray(np.asarray(inputs["bk"], np.float32)),
        "bv": np.ascontiguousarray(np.asarray(inputs["bv"], np.float32)),
        "bo": np.ascontiguousarray(
            np.asarray(inputs["bo"], np.float32).astype(NPBF16)),
    }
    half = S_FULL // 2
    maps = []
    for c in range(N_CORES):
        b, role = divmod(c, 2)
        xb = x[b]
        xp = np.concatenate([xb[role * half:(role + 1) * half],
                             xb[(1 - role) * half:(2 - role) * half]], axis=0)
        m = dict(shared)
        m["xT"] = np.ascontiguousarray(xp.T.astype(NPBF16))
        maps.append(m)
    return maps


def run(inputs, trace=False):
    nc = _get_nc()
    maps = shard_inputs(inputs)
    res = run_bass_kernel_spmd(nc, maps, list(range(N_CORES)), trace=trace)
    half = S_FULL // 2
    y = np.empty((B_FULL, S_FULL, D_FULL), dtype=np.float32)
    for c in range(N_CORES):
        b, role = divmod(c, 2)
        y[b, role * half:(role + 1) * half] = res.results[c]["y"]
    return y, res


def kernel(**inputs):
    y, _ = run(inputs, trace=False)
    return y


# revision 25
# speedup vs baseline: 1.1417x; 1.0258x over previous
"""Trainium2 Bass kernel for nn_MultiHeadAttention (B=4, S=2048, D=1024, H=16).

Sharding: 8 cores = 4 batches x 2 query-halves. Each core computes full K/V
projections for its batch (keys permuted so the core's own half comes first),
attention for its 1024 queries over all 2048 keys, and the output projection
for its query half. No collectives.

Per-core dataflow (bf16 matmuls, fp32 PSUM accumulation):
  xT arrives pre-transposed from the host ([D, S] bf16), so there is no
  on-device transpose phase. Attention runs ACT(exp)-paced (head-pair outer,
  q-span inner): per block, 16 key chunks of row-paired score matmuls ->
  exp -> ones-augmented AV matmuls. All projection work (qT/kT/v/output) is
  drip-fed just-in-time into the PE's slack via a deadline-ordered work
  queue, so the scalar engine starts exp'ing within ~12us of launch. Softmax
  denominator rides in the augmented V column; the epilogue drains PSUM to
  SBUF immediately (to unblock the next block's AV accumulation), then does
  reciprocal + K=1 broadcast matmul + bias.
"""

import numpy as np
import ml_dtypes
from collections import deque
from contextlib import ExitStack

import concourse.bass as bass
from concourse import bacc
import concourse.mybir as mybir
import concourse.tile as tile
from concourse.bass_utils import run_bass_kernel_spmd

F32 = mybir.dt.float32
BF16 = mybir.dt.bfloat16
AF = mybir.ActivationFunctionType
NPBF16 = ml_dtypes.bfloat16

P = 128

N_CORES = 8
B_FULL, S_FULL, D_FULL = 4, 2048, 1024
H_FULL, DH = 16, 64
W65 = DH + 1


def build_mha_nc(S=2048, Sq=1024, D=1024, H=16, scale=None):
    assert D % P == 0 and S % P == 0 and Sq % P == 0 and H % 2 == 0
    ND = D // P            # d-chunks (also number of head pairs)
    NS = S // P            # key chunks
    NPAIR = H // 2         # 8 head pairs
    QSP = 512              # q span per attention block
    NQS = Sq // QSP        # 2
    KSP = 512              # kT projection span
    NKS = S // KSP         # 4
    if scale is None:
        scale = DH ** -0.5

    nc = bacc.Bacc(target_bir_lowering=False, debug=False)

    xT_d = nc.dram_tensor("xT", [D, S], BF16, kind="ExternalInput").ap()
    Wq_d = nc.dram_tensor("Wq", [ND, P, ND, P], BF16, kind="ExternalInput").ap()
    Wk_d = nc.dram_tensor("Wk", [ND, P, ND, P], BF16, kind="ExternalInput").ap()
    Wv_d = nc.dram_tensor("Wv", [ND, P, ND, P], BF16, kind="ExternalInput").ap()
    Wo_d = nc.dram_tensor("Wo", [D, D], BF16, kind="ExternalInput").ap()
    bq_d = nc.dram_tensor("bq", [D], F32, kind="ExternalInput").ap()
    bk_d = nc.dram_tensor("bk", [D], F32, kind="ExternalInput").ap()
    bv_d = nc.dram_tensor("bv", [D], F32, kind="ExternalInput").ap()
    bo_d = nc.dram_tensor("bo", [D], BF16, kind="ExternalInput").ap()
    y = nc.dram_tensor("y", [Sq, D], F32, kind="ExternalOutput").ap()

    with tile.TileContext(nc) as tc, ExitStack() as top:
        top.enter_context(nc.allow_low_precision(
            reason="bf16 activations/weights with fp32 psum accumulation"))
        const = top.enter_context(tc.tile_pool(name="const", bufs=1))
        big = top.enter_context(tc.tile_pool(name="big", bufs=1))
        wkv = top.enter_context(tc.tile_pool(name="wkv", bufs=1))
        exq = top.enter_context(tc.tile_pool(name="exp", bufs=3))
        eps = top.enter_context(tc.tile_pool(name="eps", bufs=2))
        stgp = top.enter_context(tc.tile_pool(name="stgp", bufs=2))
        ystg = top.enter_context(tc.tile_pool(name="ystg", bufs=2))
        scps = top.enter_context(tc.tile_pool(name="scps", bufs=2, space="PSUM"))
        opsum = top.enter_context(tc.tile_pool(name="ops", bufs=2, space="PSUM"))
        pps = top.enter_context(tc.tile_pool(name="pps", bufs=2, space="PSUM"))

        # --- exp table warm-up: tiny activation so the ~2.7us ACT table
        # load runs during the DMA-bound preamble ---
        warm_in = const.tile([1, 8], F32)
        nc.vector.memset(warm_in, 0.0)
        warm_out = const.tile([1, 8], BF16)
        nc.scalar.activation(warm_out, warm_in, AF.Exp)

        # biases (small DMAs on the gpsimd queue)
        bq_sb = const.tile([P, ND], F32)
        nc.gpsimd.dma_start(out=bq_sb, in_=bq_d.rearrange("(j p) -> p j", p=P))
        bk_sb = const.tile([P, ND], F32)
        nc.gpsimd.dma_start(out=bk_sb, in_=bk_d.rearrange("(j p) -> p j", p=P))
        bv_even = const.tile([DH, ND], F32)
        nc.gpsimd.dma_start(
            out=bv_even, in_=bv_d.rearrange("(j q p) -> q p j", p=DH, q=2)[0])
        bv_odd = const.tile([DH, ND], F32)
        nc.gpsimd.dma_start(
            out=bv_odd, in_=bv_d.rearrange("(j q p) -> q p j", p=DH, q=2)[1])
        bo_bc = const.tile([P, D], BF16)

        # persistent SBUF
        xT = big.tile([P, ND, S], BF16)
        qTs = big.tile([P, ND, Sq], BF16)
        kp_sb = big.tile([P, NPAIR, S], BF16)
        v_sb = big.tile([P, NS, H * W65], BF16)
        oT = big.tile([P, ND, Sq], BF16)
        Wq_sb = big.tile([P, ND, D], BF16)
        Wv_sb = big.tile([P, ND, D], BF16)
        # Wk and Wo share one buffer: Wk's last use (kT pair 7) precedes
        # Wo's first use (output projection)
        Wk_sb = wkv.tile([P, ND, D], BF16, tag="w", name="Wk_sb")
        v3 = v_sb.rearrange("p i (h w) -> p i h w", w=W65)

        # softmax-denominator ones column for all chunks/heads, in one op
        nc.vector.memset(v3[:, :, :, DH:DH + 1], 1.0)

        xT_r = xT_d.rearrange("(j p) s -> p j s", p=P)

        # --- priority-ordered big DMAs; K/V weights go on the gpsimd queue
        # so they don't serialize behind xT on the sync queue ---
        nc.sync.dma_start(out=xT[:, :, 0:QSP], in_=xT_r[:, :, 0:QSP])
        nc.gpsimd.dma_start(out=Wk_sb[:, :, 0:P], in_=Wk_d[0])
        nc.sync.dma_start(out=Wq_sb[:, :, 0:P], in_=Wq_d[0])
        for s in range(4):
            nc.gpsimd.dma_start(out=Wv_sb[:, :, s * P:(s + 1) * P], in_=Wv_d[s])
        nc.sync.dma_start(out=xT[:, :, QSP:Sq], in_=xT_r[:, :, QSP:Sq])
        nc.gpsimd.dma_start(out=Wk_sb[:, :, P:2 * P], in_=Wk_d[1])
        nc.sync.dma_start(out=Wq_sb[:, :, P:2 * P], in_=Wq_d[1])
        nc.sync.dma_start(out=xT[:, :, Sq:S], in_=xT_r[:, :, Sq:S])
        for s in range(2, ND):
            nc.gpsimd.dma_start(out=Wk_sb[:, :, s * P:(s + 1) * P], in_=Wk_d[s])
            nc.sync.dma_start(out=Wq_sb[:, :, s * P:(s + 1) * P], in_=Wq_d[s])
        for s in range(4, ND):
            nc.gpsimd.dma_start(out=Wv_sb[:, :, s * P:(s + 1) * P], in_=Wv_d[s])
        nc.gpsimd.dma_start(
            out=bo_bc,
            in_=bo_d.unsqueeze(0).partition_broadcast(P).squeeze(1))

        # --- projection helpers ---
        def qT_proj(p, sp):
            ps = pps.tile([P, QSP], F32, tag="pp", name=f"qps_{p}_{sp}")
            for j in range(ND):
                nc.tensor.matmul(
                    ps,
                    Wq_sb[:, j, p * P:(p + 1) * P],
                    xT[:, j, sp * QSP:(sp + 1) * QSP],
                    start=(j == 0), stop=(j == ND - 1),
                )
            nc.vector.tensor_scalar_add(
                qTs[:, p, sp * QSP:(sp + 1) * QSP], ps, bq_sb[:, p:p + 1])

        def kT_proj(s, span):
            ps = pps.tile([P, KSP], F32, tag="pp", name=f"kps_{s}_{span}")
            for j in range(ND):
                nc.tensor.matmul(
                    ps,
                    Wk_sb[:, j, s * P:(s + 1) * P],
                    xT[:, j, span * KSP:(span + 1) * KSP],
                    start=(j == 0), stop=(j == ND - 1),
                )
            nc.vector.tensor_scalar_add(
                kp_sb[:, s, span * KSP:(span + 1) * KSP], ps, bk_sb[:, s:s + 1])

        def v_proj(i, half):
            # half covers heads 8*half..8*half+8 (v columns 512*half..)
            ps = pps.tile([P, QSP], F32, tag="pp", name=f"vps_{i}_{half}")
            for j in range(ND):
                nc.tensor.matmul(
                    ps,
                    xT[:, j, i * P:(i + 1) * P],
                    Wv_sb[:, j, half * QSP:(half + 1) * QSP],
                    start=(j == 0), stop=(j == ND - 1),
                )
            nc.vector.tensor_copy(
                v3[:, i, half * 8:(half + 1) * 8, 0:DH],
                ps.rearrange("p (h w) -> p h w", w=DH),
            )

        def wo_dma():
            # reuses Wk's buffer; allocation waits for kT pair 7 to finish
            Wo_sb = wkv.tile([P, ND, D], BF16, tag="w", name="Wo_sb")
            nc.sync.dma_start(
                out=Wo_sb, in_=Wo_d.rearrange("(j p) c -> p j c", p=P))
            return Wo_sb

        wo_holder = []

        def out_proj(sc_i, span):
            Wo_sb = wo_holder[0]
            ps = pps.tile([P, QSP], F32, tag="pp", name=f"yps_{sc_i}_{span}")
            for j in range(ND):
                nc.tensor.matmul(
                    ps,
                    oT[:, j, sc_i * P:(sc_i + 1) * P],
                    Wo_sb[:, j, span * QSP:(span + 1) * QSP],
                    start=(j == 0), stop=(j == ND - 1),
                )
            ysb = ystg.tile([P, QSP], F32, tag="ysb")
            nc.vector.tensor_add(ysb, ps, bo_bc[:, span * QSP:(span + 1) * QSP])
            nc.sync.dma_start(
                out=y[sc_i * P:(sc_i + 1) * P, span * QSP:(span + 1) * QSP],
                in_=ysb,
            )

        # --- deadline-ordered filler queue ---
        work = deque()

        def add(key, cost, fn):
            work.append((key, cost, fn))

        for i in range(1, NS):
            add(f"vA{i}", 1.85, (lambda i=i: v_proj(i, 0)))
        add("kT0s2", 1.85, lambda: kT_proj(0, 2))
        add("kT0s3", 1.85, lambda: kT_proj(0, 3))
        add("qT0s1", 1.85, lambda: qT_proj(0, 1))
        for s_ in range(NKS):
            add(f"kT1s{s_}", 1.85, (lambda s=s_: kT_proj(1, s)))
        add("qT1s0", 1.85, lambda: qT_proj(1, 0))
        add("qT1s1", 1.85, lambda: qT_proj(1, 1))
        for i in range(8):
            add(f"vB{i}", 1.85, (lambda i=i: v_proj(i, 1)))
        for s_ in range(NKS):
            add(f"kT2s{s_}", 1.85, (lambda s=s_: kT_proj(2, s)))
        add("qT2s0", 1.85, lambda: qT_proj(2, 0))
        add("qT2s1", 1.85, lambda: qT_proj(2, 1))
        for i in range(8, NS):
            add(f"vB{i}", 1.85, (lambda i=i: v_proj(i, 1)))
        for p_ in range(3, NPAIR):
            for s_ in range(NKS):
                add(f"kT{p_}s{s_}", 1.85, (lambda s=s_, q=p_: kT_proj(q, s)))
            add(f"qT{p_}s0", 1.85, (lambda q=p_: qT_proj(q, 0)))
            add(f"qT{p_}s1", 1.85, (lambda q=p_: qT_proj(q, 1)))
        add("WoDMA", 0.0, lambda: wo_holder.append(wo_dma()))
        for sc_i in range(4):
            for span in range(2):
                add(f"op0_{sc_i}_{span}", 1.85,
                    (lambda a=sc_i, b=span: out_proj(a, b)))

        emitted = set()
        op0_ok = [False]

        def pump_one():
            key, cost, fn = work.popleft()
            fn()
            emitted.add(key)
            return cost

        def pump_until(key):
            if key in emitted:
                return
            avail = {k for k, _, _ in work}
            assert key in avail, f"missing work key {key}"
            while key not in emitted:
                pump_one()

        carry = [0.0]

        def pump_credit(credit):
            carry[0] += credit
            while work and carry[0] >= work[0][1]:
                if work[0][0].startswith("op0") and not op0_ok[0]:
                    return
                carry[0] -= pump_one()

        # --- preamble projections ---
        qT_proj(0, 0)
        kT_proj(0, 0)
        kT_proj(0, 1)
        v_proj(0, 0)

        # --- attention: head pair outer, q-span inner ---
        for p in range(NPAIR):
            for sp in range(NQS):
                qsl = slice(sp * QSP, (sp + 1) * QSP)
                if not (sp == 0 and p == 0):
                    pump_until(f"qT{p}s{sp}")
                if sp == 0 and p >= 1:
                    pump_until(f"kT{p}s3")
                if sp == 0 and p >= 4:
                    pump_until(f"vB{NS - 1}")
                if sp == 0 and p == NPAIR - 1:
                    pump_until("WoDMA")
                if sp == 1 and p == NPAIR - 1:
                    op0_ok[0] = True
                o_even = opsum.tile([W65, QSP], F32, tag="op")
                o_odd = opsum.tile([W65, QSP], F32, tag="op")
                for kt in range(NS):
                    if p == 0 and sp == 0:
                        if kt >= 1:
                            pump_until(f"vA{kt}")
                        if kt == 7:
                            pump_until("kT0s2")
                        if kt == 11:
                            pump_until("kT0s3")
                    if sp == 0 and p < NPAIR - 1 and kt in (3, 7, 11):
                        pump_until(f"kT{p + 1}s{kt // 4}")
                    if sp == 0 and kt == 13:
                        pump_until(f"qT{p}s1")
                    if sp == 0 and p < NPAIR - 1 and kt == 15:
                        pump_until(f"qT{p + 1}s0")
                    sc = scps.tile([P, 2 * QSP], F32, tag="sc")
                    nc.tensor.matmul(
                        sc[:, 0:QSP],
                        kp_sb[0:DH, p, kt * P:(kt + 1) * P],
                        qTs[0:DH, p, qsl],
                        start=True, stop=True,
                    )
                    nc.tensor.matmul(
                        sc[:, QSP:2 * QSP],
                        kp_sb[DH:P, p, kt * P:(kt + 1) * P],
                        qTs[DH:P, p, qsl],
                        start=True, stop=True,
                    )
                    ex = exq.tile([P, 2 * QSP], BF16, tag="ex")
                    nc.scalar.activation(ex, sc, AF.Exp, scale=float(scale))
                    nc.tensor.matmul(
                        o_even,
                        v3[:, kt, 2 * p, :],
                        ex[:, 0:QSP],
                        start=(kt == 0), stop=(kt == NS - 1),
                    )
                    nc.tensor.matmul(
                        o_odd,
                        v3[:, kt, 2 * p + 1, :],
                        ex[:, QSP:2 * QSP],
                        start=(kt == 0), stop=(kt == NS - 1),
                    )
                    pump_credit(1.2 if (p == NPAIR - 1 and sp == 1) else 0.55)

                # epilogue: drain PSUM to SBUF early, then normalize + bias
                den_e = eps.tile([1, QSP], F32, tag="den")
                nc.vector.tensor_copy(den_e, o_even[DH:W65, :])
                stg_e = stgp.tile([DH, QSP], F32, tag="stg")
                nc.vector.tensor_copy(stg_e, o_even[0:DH, :])
                den_o = eps.tile([1, QSP], F32, tag="den")
                nc.vector.tensor_copy(den_o, o_odd[DH:W65, :])
                stg_o = stgp.tile([DH, QSP], F32, tag="stg")
                nc.vector.tensor_copy(stg_o, o_odd[0:DH, :])
                for par, stg, den in ((0, stg_e, den_e), (1, stg_o, den_o)):
                    rc0 = eps.tile([1, QSP], F32, tag="rc0")
                    nc.vector.reciprocal_approx_fast(rc0, den)
                    rc0b = eps.tile([1, QSP], BF16, tag="rc0b")
                    nc.vector.tensor_copy(rc0b, rc0)
                    rb = eps.tile([DH, QSP], BF16, tag="rb")
                    nc.gpsimd.partition_broadcast(rb, rc0b)
                    bv_sb = bv_even if par == 0 else bv_odd
                    if par == 0:
                        dst = oT[0:DH, p, qsl]
                        nc.vector.tensor_mul(dst, stg, rb)
                        nc.vector.tensor_scalar_add(dst, dst, bv_sb[:, p:p + 1])
                    else:
                        on = eps.tile([DH, QSP], BF16, tag="on")
                        nc.vector.tensor_mul(on, stg, rb)
                        nc.vector.tensor_scalar_add(on, on, bv_sb[:, p:p + 1])
                        nc.sync.dma_start(out=oT[DH:P, p, qsl], in_=on)

        # drain any remaining queued work
        while work:
            pump_one()

        # --- output projection for the second q-half ---
        for sc_i in range(4, Sq // P):
            for span in range(2):
                out_proj(sc_i, span)

    nc.compile()
    return nc


_NC = None


def _get_nc():
    global _NC
    if _NC is None:
        _NC = build_mha_nc(S=S_FULL, Sq=S_FULL // 2, D=D_FULL, H=H_FULL)
    return _NC


def _chunk_w(w, nblk=8):
    # [D, D] -> [nblk, 128, 8, 128]: block pb -> [p, j, c]
    D = w.shape[0]
    t = w.reshape(D // P, P, nblk, P)             # [j, p, pb, c]
    return np.ascontiguousarray(t.transpose(2, 1, 0, 3))


def shard_inputs(inputs):
    x = np.asarray(inputs["x"], dtype=np.float32)
    shared = {
        "Wq": _chunk_w(np.asarray(inputs["Wq"], np.float32).astype(NPBF16)),
        "Wk": _chunk_w(np.asarray(inputs["Wk"], np.float32).astype(NPBF16)),
        "Wv": _chunk_w(np.asarray(inputs["Wv"], np.float32).astype(NPBF16)),
        "Wo": np.ascontiguousarray(
            np.asarray(inputs["Wo"], np.float32).astype(NPBF16)),
        "bq": np.ascontiguousarray(np.asarray(inputs["bq"], np.float32)),
        "bk": np.ascontiguousarray(np.asarray(inputs["bk"], np.float32)),
        "bv": np.ascontiguousarray(np.asarray(inputs["bv"], np.float32)),
        "bo": np.ascontiguousarray(
            np.asarray(inputs["bo"], np.float32).astype(NPBF16)),
    }
    half = S_FULL // 2
    maps = []
    for c in range(N_CORES):
        b, role = divmod(c, 2)
        xb = x[b]
        xp = np.concatenate([xb[role * half:(role + 1) * half],
                             xb[(1 - role) * half:(2 - role) * half]], axis=0)
        m = dict(shared)
        m["xT"] = np.ascontiguousarray(xp.T.astype(NPBF16))
        maps.append(m)
    return maps


def run(inputs, trace=False):
    nc = _get_nc()
    maps = shard_inputs(inputs)
    res = run_bass_kernel_spmd(nc, maps, list(range(N_CORES)), trace=trace)
    half = S_FULL // 2
    y = np.empty((B_FULL, S_FULL, D_FULL), dtype=np.float32)
    for c in range(N_CORES):
        b, role = divmod(c, 2)
        y[b, role * half:(role + 1) * half] = res.results[c]["y"]
    return y, res


def kernel(**inputs):
    y, _ = run(inputs, trace=False)
    return y
